# revision 30
# baseline (speedup 1.0000x reference)
"""Causal single-head attention (B=4, S=4096, D=1024, fp32) on 8 TRN2 NeuronCores.

Sharding: data-parallel over batch (4) x 2-way causal-balanced query split
at 256-row granularity. Core c handles batch c//2; role r = c%2 takes the
odd (r=0) or even (r=1) global 256-row sub-blocks, packed into 4 512-col
"slots" [subA | subB] with compile-time key-chunk caps capA=32-8u /
capB=28-8u so all 8 cores run one SPMD program (72 causal units/core vs
80 for 512-granular splits); causality and per-core offsets are enforced
purely by data (mask thresholds DMA'd per core). All matmul inputs bf16:
fp32r would drop the PE clock from 2.4 to 2.0 GHz (measured 272/259 vs
216 ns per 512-wide matmul) because its 4-byte LDWEIGHTS can't hide.
No collectives (they crash this runtime: NRT_EXEC_UNIT_UNRECOVERABLE).

Per-core pipeline (all matmuls on TensorE):
  1) v = x @ Wv -> bf16, spilled to DRAM; kT = (x@Wk).T and qT = (x@Wq).T
     -> bf16, SBUF-resident. Weights double-buffered so each 2MB weight DMA
     hides under the previous projection's matmuls.
  2) per slot: scoresT[key,q] = kT-chunks.T @ qT (bf16) 512-wide for
     kc<capB then 256-wide (subA only), exp on ScalarE (scale 1/32) into a
     bf16 strip, causal mask = (iota >= thr) on VectorE per 256-half,
     denominators accumulated on VectorE + one GpSimd partition-reduce,
     out.T[e,q] accumulated in PSUM over key chunks (subB region retired
     at capB), normalized by reciprocal(sums), DMA'd out.
Host transposes x and assembles the output.
"""
import sys
import numpy as np

sys.path.insert(0, "/opt/trn_rl_repo")

B, S, D = 4, 4096, 1024
P = 128
QB = 512
QH = 256               # query sub-block (half slot)
DC = D // P            # 8 contraction chunks of 128
NSLOT = 4
MAXKC = S // P         # 32
CAPS_A = [32, 24, 16, 8]   # key-chunk cap for sub-block A (cols 0:256)
CAPS_B = [28, 20, 12, 4]   # cap for sub-block B (cols 256:512)
NCORES = 8
QLOC = NSLOT * QB      # 2048 query rows per core
SCALE = 1.0 / np.sqrt(np.float32(D))     # softmax 1/sqrt(d_out)


def _sub_block(role, u, half):
    """Global 256-row sub-block index for (role, slot u, half A/B)."""
    return (15 if half == 0 else 13) - 4 * u - role

_built = None


def _build():
    import concourse.mybir as mybir
    import concourse.tile as tile
    from concourse import bacc
    from concourse import bass_isa

    f32 = mybir.dt.float32
    bf16 = mybir.dt.bfloat16

    nc = bacc.Bacc("TRN2", target_bir_lowering=False, debug=False,
                   num_devices=NCORES)
    xT = nc.dram_tensor("xT", [D, S], bf16, kind="ExternalInput")
    xTq = nc.dram_tensor("xTq", [D, QLOC], bf16, kind="ExternalInput")
    Wq = nc.dram_tensor("Wq", [D, D], bf16, kind="ExternalInput")
    Wk = nc.dram_tensor("Wk", [D, D], bf16, kind="ExternalInput")
    Wv = nc.dram_tensor("Wv", [D, D], bf16, kind="ExternalInput")
    thrA = nc.dram_tensor("thrA", [P, NSLOT * MAXKC], f32,
                          kind="ExternalInput")
    thrB = nc.dram_tensor("thrB", [P, NSLOT * MAXKC], f32,
                          kind="ExternalInput")
    iota = nc.dram_tensor("iota", [P, QH], f32, kind="ExternalInput")
    outT = nc.dram_tensor("outT", [D, QLOC], f32, kind="ExternalOutput")

    xT_r = xT.ap().rearrange("(c p) s -> p c s", p=P)
    xTq_r = xTq.ap().rearrange("(c p) s -> p c s", p=P)
    W_r = {"q": Wq.ap().rearrange("(c p) e -> p c e", p=P),
           "k": Wk.ap().rearrange("(c p) e -> p c e", p=P),
           "v": Wv.ap().rearrange("(c p) e -> p c e", p=P)}

    with tile.TileContext(nc) as tc, \
         tc.tile_pool(name="res", bufs=1) as res, \
         tc.tile_pool(name="const", bufs=1) as constp, \
         tc.tile_pool(name="p1small", bufs=3) as p1small, \
         tc.tile_pool(name="dram", bufs=1, space="DRAM") as dramp, \
         tc.tile_pool(name="psA", bufs=6, space="PSUM") as psA, \
         tc.tile_pool(name="psS", bufs=2, space="PSUM") as psS:

        kT = res.tile([P, DC, S], bf16, tag="kT")
        qT = res.tile([P, DC, QLOC], bf16, tag="qT")
        vsp = dramp.tile([S, D], bf16, tag="vsp")

        iota_sb = constp.tile([P, QH], f32, tag="iota")
        thrA_sb = constp.tile([P, NSLOT * MAXKC], f32, tag="thrA")
        thrB_sb = constp.tile([P, NSLOT * MAXKC], f32, tag="thrB")
        nc.sync.dma_start(out=iota_sb[:], in_=iota.ap())
        nc.sync.dma_start(out=thrA_sb[:], in_=thrA.ap())
        nc.sync.dma_start(out=thrB_sb[:], in_=thrB.ap())

        # ---------------- phase 1: projections (fp32r) ----------------
        # Order: qT (Wq) -> fused kT+v sweep over xT (Wk, Wv). Weight DMAs
        # are split per 128-col slice and deferred so the lead q-strip +
        # Wq's first slices get the DMA bandwidth at kernel start; Wk
        # loads during qT, Wv during the first kT block. kT and v share
        # one x-strip load per 512-column block of xT.
        with tc.tile_pool(name="wa", bufs=1) as wa, \
             tc.tile_pool(name="wb", bufs=1) as wb, \
             tc.tile_pool(name="xs", bufs=2) as xs:

            def load_w(pool, which, nm, ec0=0):
                w_sb = pool.tile([P, DC, D], bf16, tag=pool.name, name=nm)
                for ec in range(ec0, DC):
                    nc.sync.dma_start(
                        out=w_sb[:, :, ec * P:(ec + 1) * P],
                        in_=W_r[which][:, :, ec * P:(ec + 1) * P])
                return w_sb

            def load_xstrip(src_r, blk, nm):
                xstrip = xs.tile([P, DC, QB], bf16, tag="xs", name=nm)
                for dc in range(DC):
                    nc.sync.dma_start(
                        out=xstrip[:, dc],
                        in_=src_r[:, dc, blk * QB:(blk + 1) * QB])
                return xstrip

            # DMA order at kernel start: the ec=0 slice of Wq (256KB) so
            # the first chain's LDWEIGHTS unblocks ASAP, then the lead
            # q-strip, then the Wq bulk.
            wq_sb = wa.tile([P, DC, D], bf16, tag=wa.name, name="wq_sb")
            nc.sync.dma_start(out=wq_sb[:, :, 0:P], in_=W_r["q"][:, :, 0:P])
            xstrip0 = load_xstrip(xTq_r, 0, "xq_0")
            for ec in range(1, DC):
                nc.sync.dma_start(
                    out=wq_sb[:, :, ec * P:(ec + 1) * P],
                    in_=W_r["q"][:, :, ec * P:(ec + 1) * P])
            wk_sb = None

            # qT = (x_q @ Wq).T
            for blk in range(QLOC // QB):
                xstrip = xstrip0 if blk == 0 else \
                    load_xstrip(xTq_r, blk, f"xq_{blk}")
                if blk == 1:
                    # defer the Wk DMA off the kernel-start critical path
                    wk_sb = load_w(wb, "k", "wk_sb")
                for ec in range(DC):
                    pp = psA if ec % 2 == 0 else psS
                    acc = pp.tile([P, QB], f32,
                                  tag="acc" if ec % 2 == 0 else "sc",
                                  name=f"qacc_{blk}_{ec}")
                    for dc in range(DC):
                        nc.tensor.matmul(
                            acc[:],
                            lhsT=wq_sb[:, dc, ec * P:(ec + 1) * P],
                            rhs=xstrip[:, dc],
                            start=(dc == 0), stop=(dc == DC - 1))
                    d = qT[:, ec, blk * QB:(blk + 1) * QB]
                    if ec % 2 == 0:
                        nc.vector.tensor_copy(d, acc[:])
                    else:
                        nc.scalar.copy(d, acc[:])

            # fused kT + v sweep (one x-strip per block feeds both);
            # Wv reuses Wq's slot, its DMA hides under the first kT block
            wv_sb = load_w(wa, "v", "wv_sb")
            for blk in range(S // QB):
                xstrip = load_xstrip(xT_r, blk, f"xkv_{blk}")
                for ec in range(DC):
                    pp = psA if ec % 2 == 0 else psS
                    acc = pp.tile([P, QB], f32,
                                  tag="acc" if ec % 2 == 0 else "sc",
                                  name=f"kacc_{blk}_{ec}")
                    for dc in range(DC):
                        nc.tensor.matmul(
                            acc[:],
                            lhsT=wk_sb[:, dc, ec * P:(ec + 1) * P],
                            rhs=xstrip[:, dc],
                            start=(dc == 0), stop=(dc == DC - 1))
                    d = kT[:, ec, blk * QB:(blk + 1) * QB]
                    if ec % 2 == 0:
                        nc.vector.tensor_copy(d, acc[:])
                    else:
                        nc.scalar.copy(d, acc[:])
                for ss in range(QB // P):
                    for eb in range(D // QB):
                        pp = psA if (ss + eb) % 2 == 0 else psS
                        acc = pp.tile([P, QB], f32,
                                      tag="acc" if (ss + eb) % 2 == 0
                                      else "sc",
                                      name=f"vacc_{blk}_{ss}_{eb}")
                        for dc in range(DC):
                            nc.tensor.matmul(
                                acc[:],
                                lhsT=xstrip[:, dc, ss * P:(ss + 1) * P],
                                rhs=wv_sb[:, dc, eb * QB:(eb + 1) * QB],
                                start=(dc == 0), stop=(dc == DC - 1))
                        vtmp = p1small.tile([P, QB], bf16, tag="vtmp",
                                            name=f"vtmp_{blk}_{ss}_{eb}")
                        if (ss + eb) % 2 == 0:
                            nc.vector.tensor_copy(vtmp[:], acc[:])
                        else:
                            nc.scalar.copy(vtmp[:], acc[:])
                        r0 = blk * QB + ss * P
                        nc.sync.dma_start(
                            out=vsp[r0:r0 + P, eb * QB:(eb + 1) * QB],
                            in_=vtmp[:])

        # ---------------- phase 2: attention ----------------
        # Slot u = 512 q cols = [subA (0:256) | subB (256:512)], two
        # 256-row sub-blocks with key-chunk needs capA=32-8u / capB=28-8u.
        # Scores run 512-wide for kc<capB, then 256-wide (subA only) for
        # kc in [capB, capA); AV likewise skips the dead subB region.
        # This realizes the 256-granular causal balance (72 units/core vs
        # 80) while keeping 512-wide matmuls on the bulk.
        with tc.tile_pool(name="expp", bufs=2) as expp, \
             tc.tile_pool(name="vs", bufs=12) as vs, \
             tc.tile_pool(name="p2small", bufs=3) as p2s:
            # biggest slots first; end on cap=24 so the final slot's
            # GpSimd-reduce + reciprocal chain hides under its out.T
            # accumulation
            for u in (0, 2, 3, 1):
                capA, capB = CAPS_A[u], CAPS_B[u]
                expT = expp.tile([P, MAXKC, QB], bf16, tag="expT",
                                 name=f"expT_{u}")
                # scoresT -> exp -> mask; per-partition partial sums
                # accumulate on VectorE (fp32) as tiles arrive, then one
                # GpSimd partition_all_reduce gives the softmax
                # denominators without spending TensorE matmuls.
                sacc = p2s.tile([P, QB], f32, tag="sacc", name=f"sacc_{u}")

                def mask_half(col0, thr_sb_, kc, nm):
                    m = p2s.tile([P, QH], bf16, tag="mask", name=nm)
                    nc.vector.tensor_scalar(
                        m[:], iota_sb[:],
                        thr_sb_[:, u * MAXKC + kc:u * MAXKC + kc + 1],
                        None, mybir.AluOpType.is_ge)
                    nc.vector.tensor_mul(expT[:, kc, col0:col0 + QH],
                                         expT[:, kc, col0:col0 + QH], m[:])

                for kc in range(capA):
                    wide = QB if kc < capB else QH
                    sc = psS.tile([P, QB], f32, tag="sc",
                                  name=f"sc_{u}_{kc}")
                    for ec in range(DC):
                        nc.tensor.matmul(
                            sc[:, 0:wide],
                            lhsT=kT[:, ec, kc * P:(kc + 1) * P],
                            rhs=qT[:, ec, u * QB:u * QB + wide],
                            start=(ec == 0), stop=(ec == DC - 1))
                    nc.scalar.activation(
                        expT[:, kc, 0:wide], sc[:, 0:wide],
                        func=mybir.ActivationFunctionType.Exp,
                        scale=float(SCALE))
                    if kc >= capB:
                        mask_half(0, thrA_sb, kc, f"mA_{u}_{kc}")
                    elif kc >= capB - 4:
                        mask_half(QH, thrB_sb, kc, f"mB_{u}_{kc}")
                    if kc == 0:
                        nc.vector.tensor_copy(sacc[:], expT[:, 0])
                    else:
                        nc.vector.tensor_add(
                            sacc[:, 0:wide], sacc[:, 0:wide],
                            expT[:, kc, 0:wide])
                sums_sb = p2s.tile([P, QB], f32, tag="sums",
                                   name=f"sums_{u}")
                nc.gpsimd.partition_all_reduce(
                    sums_sb[:], sacc[:], P, bass_isa.ReduceOp.add)
                recip = p2s.tile([P, QB], f32, tag="recip",
                                 name=f"recip_{u}")
                nc.vector.reciprocal(recip[:], sums_sb[:])
                # out.T accumulation, e in two halves of 4 chunks; subB's
                # accumulation region stops at capB-1, subA's at capA-1
                for half in range(2):
                    accs = [psA.tile([P, QB], f32, tag="acc",
                                     name=f"oacc_{u}_{half}_{i}")
                            for i in range(4)]
                    for kc in range(capA):
                        vh = vs.tile([P, QB], bf16, tag="vh",
                                     name=f"vh_{u}_{half}_{kc}")
                        nc.sync.dma_start(
                            out=vh[:],
                            in_=vsp[kc * P:(kc + 1) * P,
                                    half * QB:(half + 1) * QB])
                        for e4 in range(4):
                            lw = vh[:, e4 * P:(e4 + 1) * P]
                            if kc < capB - 1:
                                nc.tensor.matmul(
                                    accs[e4][:], lhsT=lw, rhs=expT[:, kc],
                                    start=(kc == 0), stop=False)
                            elif kc == capB - 1:
                                nc.tensor.matmul(
                                    accs[e4][:, 0:QH], lhsT=lw,
                                    rhs=expT[:, kc, 0:QH],
                                    start=False, stop=False,
                                    skip_group_check=True)
                                nc.tensor.matmul(
                                    accs[e4][:, QH:QB], lhsT=lw,
                                    rhs=expT[:, kc, QH:QB],
                                    start=False, stop=True,
                                    skip_group_check=True)
                            else:
                                nc.tensor.matmul(
                                    accs[e4][:, 0:QH], lhsT=lw,
                                    rhs=expT[:, kc, 0:QH],
                                    start=False, stop=(kc == capA - 1),
                                    skip_group_check=True)
                    for e4 in range(4):
                        # normalize straight out of PSUM (recip is ready
                        # well before the accumulation ends), then DMA
                        ot = p2s.tile([P, QB], f32, tag="ot",
                                      name=f"ot_{u}_{half}_{e4}")
                        nc.vector.tensor_mul(ot[:], accs[e4][:], recip[:])
                        r0 = (half * 4 + e4) * P
                        nc.sync.dma_start(
                            out=outT.ap()[r0:r0 + P, u * QB:(u + 1) * QB],
                            in_=ot[:])

    nc.finalize()
    return nc


def _get_nc():
    global _built
    if _built is None:
        _built = _build()
    return _built


def _host_inputs(x, Wq, Wk, Wv):
    import ml_dtypes
    bf16 = ml_dtypes.bfloat16
    iota = np.broadcast_to(
        np.arange(QH, dtype=np.float32), (P, QH)).copy()
    Wq = np.ascontiguousarray(np.asarray(Wq, dtype=np.float32).astype(bf16))
    Wk = np.ascontiguousarray(np.asarray(Wk, dtype=np.float32).astype(bf16))
    Wv = np.ascontiguousarray(np.asarray(Wv, dtype=np.float32).astype(bf16))
    p = np.arange(P, dtype=np.float32)
    thrAs, thrBs = [], []
    for role in range(2):
        tA = np.zeros((P, NSLOT * MAXKC), np.float32)
        tB = np.zeros((P, NSLOT * MAXKC), np.float32)
        for u in range(NSLOT):
            qA = QH * _sub_block(role, u, 0)
            qB = QH * _sub_block(role, u, 1)
            for kc in range(MAXKC):
                tA[:, u * MAXKC + kc] = np.clip(kc * P + p - qA, 0, QH)
                tB[:, u * MAXKC + kc] = np.clip(kc * P + p - qB, 0, QH)
        thrAs.append(tA)
        thrBs.append(tB)
    xTs = [np.ascontiguousarray(np.asarray(x[b]).T.astype(bf16))
           for b in range(B)]
    in_maps = []
    for c in range(NCORES):
        b, role = divmod(c, 2)
        cols = np.concatenate(
            [np.arange(QH * _sub_block(role, u, h),
                       QH * _sub_block(role, u, h) + QH)
             for u in range(NSLOT) for h in range(2)])
        xTq = np.ascontiguousarray(xTs[b][:, cols])
        in_maps.append({"xT": xTs[b], "xTq": xTq, "Wq": Wq, "Wk": Wk,
                        "Wv": Wv, "thrA": thrAs[role], "thrB": thrBs[role],
                        "iota": iota})
    return in_maps


def _assemble(results):
    out = np.empty((B, S, D), np.float32)
    for c in range(NCORES):
        b, role = divmod(c, 2)
        oT = results[c]["outT"]
        for u in range(NSLOT):
            for h in range(2):
                q0 = QH * _sub_block(role, u, h)
                c0 = u * QB + h * QH
                out[b, q0:q0 + QH, :] = oT[:, c0:c0 + QH].T
    return out


def run_cores(in_maps, trace=False):
    from concourse.bass_utils import run_bass_kernel_spmd
    nc = _get_nc()
    return run_bass_kernel_spmd(nc, in_maps, list(range(NCORES)), trace=trace)


def kernel(x, Wq, Wk, Wv):
    x = np.asarray(x, dtype=np.float32)
    in_maps = _host_inputs(x, Wq, Wk, Wv)
    res = None
    for attempt in range(3):
        try:
            res = run_cores(in_maps, trace=False)
            break
        except Exception:
            # retries absorb transient device-unrecoverable blips
            if attempt == 2:
                raise
    return _assemble(res.results)



# revision 31
# speedup vs baseline: 1.2025x; 1.2025x over previous
"""Causal single-head attention (B=4, S=4096, D=1024, fp32) on 8 TRN2 NeuronCores.

Sharding: data-parallel over batch (4) x 2-way causal-balanced query split
at 256-row granularity. Core c handles batch c//2; role r = c%2 takes the
odd (r=0) or even (r=1) global 256-row sub-blocks, packed into 4 512-col
"slots" [subA | subB] with compile-time key-chunk caps capA=32-8u /
capB=28-8u so all 8 cores run one SPMD program (72 causal units/core vs
80 for 512-granular splits); causality and per-core offsets are enforced
purely by data (mask thresholds DMA'd per core). All matmul inputs bf16:
fp32r would drop the PE clock from 2.4 to 2.0 GHz (measured 272/259 vs
216 ns per 512-wide matmul) because its 4-byte LDWEIGHTS can't hide.
No collectives (they crash this runtime: NRT_EXEC_UNIT_UNRECOVERABLE).

Per-core pipeline (all matmuls on TensorE):
  1) v = x @ Wv -> bf16, spilled to DRAM; kT = (x@Wk).T and qT = (x@Wq).T
     -> bf16, SBUF-resident. Weights double-buffered so each 2MB weight DMA
     hides under the previous projection's matmuls.
  2) per slot: scoresT[key,q] = kT-chunks.T @ qT (bf16) 512-wide for
     kc<capB then 256-wide (subA only), exp on ScalarE (scale 1/32) into a
     bf16 strip, causal mask = (iota >= thr) on VectorE per 256-half,
     denominators accumulated on VectorE + one GpSimd partition-reduce,
     out.T[e,q] accumulated in PSUM over key chunks (subB region retired
     at capB), normalized by reciprocal(sums), DMA'd out.
Host transposes x and assembles the output.
"""
import sys
import numpy as np

sys.path.insert(0, "/opt/trn_rl_repo")

B, S, D = 4, 4096, 1024
P = 128
QB = 512
QH = 128               # query sub-block (quarter slot)
DC = D // P            # 8 contraction chunks of 128
NSLOT = 4
MAXKC = S // P         # 32
# quarter c of slot u (cols [128c:128c+128]) holds the 128-row sub-block
# needing cap 32-8u-2c key chunks; score width shrinks along the diagonal
CAPS = [[32 - 8 * u - 2 * c for c in range(4)] for u in range(4)]
NCORES = 8
QLOC = NSLOT * QB      # 2048 query rows per core
SCALE = 1.0 / np.sqrt(np.float32(D))     # softmax 1/sqrt(d_out)


def _sub_block(role, u, c):
    """Global 128-row sub-block index for (role, slot u, quarter c)."""
    return 31 - 8 * u - 2 * c - role

_built = None


def _build():
    import concourse.mybir as mybir
    import concourse.tile as tile
    from concourse import bacc
    from concourse import bass_isa

    f32 = mybir.dt.float32
    bf16 = mybir.dt.bfloat16

    nc = bacc.Bacc("TRN2", target_bir_lowering=False, debug=False,
                   num_devices=NCORES)
    xT = nc.dram_tensor("xT", [D, S], bf16, kind="ExternalInput")
    xTq = nc.dram_tensor("xTq", [D, QLOC], bf16, kind="ExternalInput")
    Wq = nc.dram_tensor("Wq", [D, D], bf16, kind="ExternalInput")
    Wk = nc.dram_tensor("Wk", [D, D], bf16, kind="ExternalInput")
    Wv = nc.dram_tensor("Wv", [D, D], bf16, kind="ExternalInput")
    thrs = [nc.dram_tensor(f"thr{c}", [P, NSLOT * MAXKC], f32,
                           kind="ExternalInput") for c in range(4)]
    iota = nc.dram_tensor("iota", [P, QH], f32, kind="ExternalInput")
    outT = nc.dram_tensor("outT", [D, QLOC], f32, kind="ExternalOutput")

    xT_r = xT.ap().rearrange("(c p) s -> p c s", p=P)
    xTq_r = xTq.ap().rearrange("(c p) s -> p c s", p=P)
    W_r = {"q": Wq.ap().rearrange("(c p) e -> p c e", p=P),
           "k": Wk.ap().rearrange("(c p) e -> p c e", p=P),
           "v": Wv.ap().rearrange("(c p) e -> p c e", p=P)}

    with tile.TileContext(nc) as tc, \
         tc.tile_pool(name="res", bufs=1) as res, \
         tc.tile_pool(name="const", bufs=1) as constp, \
         tc.tile_pool(name="p1small", bufs=3) as p1small, \
         tc.tile_pool(name="dram", bufs=1, space="DRAM") as dramp, \
         tc.tile_pool(name="psA", bufs=6, space="PSUM") as psA, \
         tc.tile_pool(name="psS", bufs=2, space="PSUM") as psS:

        kT = res.tile([P, DC, S], bf16, tag="kT")
        qT = res.tile([P, DC, QLOC], bf16, tag="qT")
        vsp = dramp.tile([S, D], bf16, tag="vsp")

        iota_sb = constp.tile([P, QH], f32, tag="iota")
        thr_sbs = [constp.tile([P, NSLOT * MAXKC], f32, tag=f"thr{c}",
                               name=f"thr{c}_sb") for c in range(4)]
        nc.sync.dma_start(out=iota_sb[:], in_=iota.ap())
        for c in range(4):
            nc.sync.dma_start(out=thr_sbs[c][:], in_=thrs[c].ap())

        # ---------------- phase 1: projections (fp32r) ----------------
        # Order: qT (Wq) -> fused kT+v sweep over xT (Wk, Wv). Weight DMAs
        # are split per 128-col slice and deferred so the lead q-strip +
        # Wq's first slices get the DMA bandwidth at kernel start; Wk
        # loads during qT, Wv during the first kT block. kT and v share
        # one x-strip load per 512-column block of xT.
        with tc.tile_pool(name="wa", bufs=1) as wa, \
             tc.tile_pool(name="wb", bufs=1) as wb, \
             tc.tile_pool(name="xs", bufs=2) as xs:

            def load_w(pool, which, nm, ec0=0):
                w_sb = pool.tile([P, DC, D], bf16, tag=pool.name, name=nm)
                for ec in range(ec0, DC):
                    nc.sync.dma_start(
                        out=w_sb[:, :, ec * P:(ec + 1) * P],
                        in_=W_r[which][:, :, ec * P:(ec + 1) * P])
                return w_sb

            def load_xstrip(src_r, blk, nm):
                xstrip = xs.tile([P, DC, QB], bf16, tag="xs", name=nm)
                for dc in range(DC):
                    nc.sync.dma_start(
                        out=xstrip[:, dc],
                        in_=src_r[:, dc, blk * QB:(blk + 1) * QB])
                return xstrip

            # DMA order at kernel start: the ec=0 slice of Wq (256KB) so
            # the first chain's LDWEIGHTS unblocks ASAP, then the lead
            # q-strip, then the Wq bulk.
            wq_sb = wa.tile([P, DC, D], bf16, tag=wa.name, name="wq_sb")
            nc.sync.dma_start(out=wq_sb[:, :, 0:P], in_=W_r["q"][:, :, 0:P])
            xstrip0 = load_xstrip(xTq_r, 0, "xq_0")
            for ec in range(1, DC):
                nc.sync.dma_start(
                    out=wq_sb[:, :, ec * P:(ec + 1) * P],
                    in_=W_r["q"][:, :, ec * P:(ec + 1) * P])
            wk_sb = None

            # qT = (x_q @ Wq).T
            for blk in range(QLOC // QB):
                xstrip = xstrip0 if blk == 0 else \
                    load_xstrip(xTq_r, blk, f"xq_{blk}")
                if blk == 1:
                    # defer the Wk DMA off the kernel-start critical path
                    wk_sb = load_w(wb, "k", "wk_sb")
                for ec in range(DC):
                    pp = psA if ec % 2 == 0 else psS
                    acc = pp.tile([P, QB], f32,
                                  tag="acc" if ec % 2 == 0 else "sc",
                                  name=f"qacc_{blk}_{ec}")
                    for dc in range(DC):
                        nc.tensor.matmul(
                            acc[:],
                            lhsT=wq_sb[:, dc, ec * P:(ec + 1) * P],
                            rhs=xstrip[:, dc],
                            start=(dc == 0), stop=(dc == DC - 1))
                    d = qT[:, ec, blk * QB:(blk + 1) * QB]
                    if ec % 2 == 0:
                        nc.vector.tensor_copy(d, acc[:])
                    else:
                        nc.scalar.copy(d, acc[:])

            # fused kT + v sweep (one x-strip per block feeds both);
            # Wv reuses Wq's slot, its DMA hides under the first kT block
            wv_sb = load_w(wa, "v", "wv_sb")
            for blk in range(S // QB):
                xstrip = load_xstrip(xT_r, blk, f"xkv_{blk}")
                for ec in range(DC):
                    pp = psA if ec % 2 == 0 else psS
                    acc = pp.tile([P, QB], f32,
                                  tag="acc" if ec % 2 == 0 else "sc",
                                  name=f"kacc_{blk}_{ec}")
                    for dc in range(DC):
                        nc.tensor.matmul(
                            acc[:],
                            lhsT=wk_sb[:, dc, ec * P:(ec + 1) * P],
                            rhs=xstrip[:, dc],
                            start=(dc == 0), stop=(dc == DC - 1))
                    d = kT[:, ec, blk * QB:(blk + 1) * QB]
                    if ec % 2 == 0:
                        nc.vector.tensor_copy(d, acc[:])
                    else:
                        nc.scalar.copy(d, acc[:])
                for ss in range(QB // P):
                    for eb in range(D // QB):
                        pp = psA if (ss + eb) % 2 == 0 else psS
                        acc = pp.tile([P, QB], f32,
                                      tag="acc" if (ss + eb) % 2 == 0
                                      else "sc",
                                      name=f"vacc_{blk}_{ss}_{eb}")
                        for dc in range(DC):
                            nc.tensor.matmul(
                                acc[:],
                                lhsT=xstrip[:, dc, ss * P:(ss + 1) * P],
                                rhs=wv_sb[:, dc, eb * QB:(eb + 1) * QB],
                                start=(dc == 0), stop=(dc == DC - 1))
                        vtmp = p1small.tile([P, QB], bf16, tag="vtmp",
                                            name=f"vtmp_{blk}_{ss}_{eb}")
                        if (ss + eb) % 2 == 0:
                            nc.vector.tensor_copy(vtmp[:], acc[:])
                        else:
                            nc.scalar.copy(vtmp[:], acc[:])
                        r0 = blk * QB + ss * P
                        nc.sync.dma_start(
                            out=vsp[r0:r0 + P, eb * QB:(eb + 1) * QB],
                            in_=vtmp[:])

        # ---------------- phase 2: attention ----------------
        # Slot u = 512 q cols = [subA (0:256) | subB (256:512)], two
        # 256-row sub-blocks with key-chunk needs capA=32-8u / capB=28-8u.
        # Scores run 512-wide for kc<capB, then 256-wide (subA only) for
        # kc in [capB, capA); AV likewise skips the dead subB region.
        # This realizes the 256-granular causal balance (72 units/core vs
        # 80) while keeping 512-wide matmuls on the bulk.
        with tc.tile_pool(name="expp", bufs=2) as expp, \
             tc.tile_pool(name="vs", bufs=12) as vs, \
             tc.tile_pool(name="p2small", bufs=3) as p2s:
            # biggest slots first; end on cap=24 so the final slot's
            # GpSimd-reduce + reciprocal chain hides under its out.T
            # accumulation
            for u in (0, 2, 3, 1):
                caps = CAPS[u]
                capA = caps[0]
                expT = expp.tile([P, MAXKC, QB], bf16, tag="expT",
                                 name=f"expT_{u}")
                # scoresT -> exp -> mask; per-partition partial sums
                # accumulate on VectorE (fp32) as tiles arrive, then one
                # GpSimd partition_all_reduce gives the softmax
                # denominators without spending TensorE matmuls.
                sacc = p2s.tile([P, QB], f32, tag="sacc", name=f"sacc_{u}")

                def mask_q(c, kc, nm):
                    m = p2s.tile([P, QH], bf16, tag="mask", name=nm)
                    nc.vector.tensor_scalar(
                        m[:], iota_sb[:],
                        thr_sbs[c][:, u * MAXKC + kc:u * MAXKC + kc + 1],
                        None, mybir.AluOpType.is_ge)
                    nc.vector.tensor_mul(expT[:, kc, c * QH:(c + 1) * QH],
                                         expT[:, kc, c * QH:(c + 1) * QH],
                                         m[:])

                for kc in range(capA):
                    wide = QH * sum(1 for c in range(4) if kc < caps[c])
                    sc = psS.tile([P, QB], f32, tag="sc",
                                  name=f"sc_{u}_{kc}")
                    for ec in range(DC):
                        nc.tensor.matmul(
                            sc[:, 0:wide],
                            lhsT=kT[:, ec, kc * P:(kc + 1) * P],
                            rhs=qT[:, ec, u * QB:u * QB + wide],
                            start=(ec == 0), stop=(ec == DC - 1))
                    nc.scalar.activation(
                        expT[:, kc, 0:wide], sc[:, 0:wide],
                        func=mybir.ActivationFunctionType.Exp,
                        scale=float(SCALE))
                    for c in range(4):
                        if caps[c] - 2 <= kc < caps[c]:
                            mask_q(c, kc, f"m{c}_{u}_{kc}")
                    if kc == 0:
                        nc.vector.tensor_copy(sacc[:], expT[:, 0])
                    else:
                        nc.vector.tensor_add(
                            sacc[:, 0:wide], sacc[:, 0:wide],
                            expT[:, kc, 0:wide])
                sums_sb = p2s.tile([P, QB], f32, tag="sums",
                                   name=f"sums_{u}")
                nc.gpsimd.partition_all_reduce(
                    sums_sb[:], sacc[:], P, bass_isa.ReduceOp.add)
                recip = p2s.tile([P, QB], f32, tag="recip",
                                 name=f"recip_{u}")
                nc.vector.reciprocal(recip[:], sums_sb[:])
                # out.T accumulation, e in two halves of 4 chunks; subB's
                # accumulation region stops at capB-1, subA's at capA-1
                for half in range(2):
                    accs = [psA.tile([P, QB], f32, tag="acc",
                                     name=f"oacc_{u}_{half}_{i}")
                            for i in range(4)]
                    for kc in range(capA):
                        vh = vs.tile([P, QB], bf16, tag="vh",
                                     name=f"vh_{u}_{half}_{kc}")
                        nc.sync.dma_start(
                            out=vh[:],
                            in_=vsp[kc * P:(kc + 1) * P,
                                    half * QB:(half + 1) * QB])
                        wide = QH * sum(1 for c in range(4)
                                        if kc < caps[c])
                        stopc = [c for c in range(4) if kc == caps[c] - 1]
                        for e4 in range(4):
                            lw = vh[:, e4 * P:(e4 + 1) * P]
                            if stopc:
                                c = stopc[0]
                                if c > 0:
                                    nc.tensor.matmul(
                                        accs[e4][:, 0:c * QH], lhsT=lw,
                                        rhs=expT[:, kc, 0:c * QH],
                                        start=False, stop=False,
                                        skip_group_check=True)
                                nc.tensor.matmul(
                                    accs[e4][:, c * QH:(c + 1) * QH],
                                    lhsT=lw,
                                    rhs=expT[:, kc, c * QH:(c + 1) * QH],
                                    start=False, stop=True,
                                    skip_group_check=True)
                            else:
                                nc.tensor.matmul(
                                    accs[e4][:, 0:wide], lhsT=lw,
                                    rhs=expT[:, kc, 0:wide],
                                    start=(kc == 0), stop=False,
                                    skip_group_check=(wide != QB))
                    for e4 in range(4):
                        # normalize straight out of PSUM (recip is ready
                        # well before the accumulation ends), then DMA
                        ot = p2s.tile([P, QB], f32, tag="ot",
                                      name=f"ot_{u}_{half}_{e4}")
                        nc.vector.tensor_mul(ot[:], accs[e4][:], recip[:])
                        r0 = (half * 4 + e4) * P
                        nc.sync.dma_start(
                            out=outT.ap()[r0:r0 + P, u * QB:(u + 1) * QB],
                            in_=ot[:])

    nc.finalize()
    return nc


def _get_nc():
    global _built
    if _built is None:
        _built = _build()
    return _built


def _host_inputs(x, Wq, Wk, Wv):
    import ml_dtypes
    bf16 = ml_dtypes.bfloat16
    iota = np.broadcast_to(
        np.arange(QH, dtype=np.float32), (P, QH)).copy()
    Wq = np.ascontiguousarray(np.asarray(Wq, dtype=np.float32).astype(bf16))
    Wk = np.ascontiguousarray(np.asarray(Wk, dtype=np.float32).astype(bf16))
    Wv = np.ascontiguousarray(np.asarray(Wv, dtype=np.float32).astype(bf16))
    p = np.arange(P, dtype=np.float32)
    thr_tabs = []
    for role in range(2):
        ts = [np.zeros((P, NSLOT * MAXKC), np.float32) for _ in range(4)]
        for u in range(NSLOT):
            for c in range(4):
                q0 = QH * _sub_block(role, u, c)
                for kc in range(MAXKC):
                    ts[c][:, u * MAXKC + kc] = np.clip(
                        kc * P + p - q0, 0, QH)
        thr_tabs.append(ts)
    xTs = [np.ascontiguousarray(np.asarray(x[b]).T.astype(bf16))
           for b in range(B)]
    in_maps = []
    for c in range(NCORES):
        b, role = divmod(c, 2)
        cols = np.concatenate(
            [np.arange(QH * _sub_block(role, u, c),
                       QH * _sub_block(role, u, c) + QH)
             for u in range(NSLOT) for c in range(4)])
        xTq = np.ascontiguousarray(xTs[b][:, cols])
        im = {"xT": xTs[b], "xTq": xTq, "Wq": Wq, "Wk": Wk,
              "Wv": Wv, "iota": iota}
        for c in range(4):
            im[f"thr{c}"] = thr_tabs[role][c]
        in_maps.append(im)
    return in_maps


def _assemble(results):
    out = np.empty((B, S, D), np.float32)
    for c in range(NCORES):
        b, role = divmod(c, 2)
        oT = results[c]["outT"]
        for u in range(NSLOT):
            for c in range(4):
                q0 = QH * _sub_block(role, u, c)
                c0 = u * QB + c * QH
                out[b, q0:q0 + QH, :] = oT[:, c0:c0 + QH].T
    return out


def run_cores(in_maps, trace=False):
    from concourse.bass_utils import run_bass_kernel_spmd
    nc = _get_nc()
    return run_bass_kernel_spmd(nc, in_maps, list(range(NCORES)), trace=trace)


def kernel(x, Wq, Wk, Wv):
    x = np.asarray(x, dtype=np.float32)
    in_maps = _host_inputs(x, Wq, Wk, Wv)
    res = None
    for attempt in range(3):
        try:
            res = run_cores(in_maps, trace=False)
            break
        except Exception:
            # retries absorb transient device-unrecoverable blips
            if attempt == 2:
                raise
    return _assemble(res.results)

